# revision 30
# baseline (speedup 1.0000x reference)
"""Trainium2 Bass kernel for nn_Mlp_13099650253522 (BitNet-ternary dense MLP).

  h = gelu(x @ ter_quant(w1).T + b1);  y = h @ ter_quant(w2).T + b2
  ter_quant(w) = clip(round(w / g), -1, 1) * g,  g = mean(|w|) + 1e-5

Strategy (8 NeuronCores, data-parallel over the 64*197=12608 tokens):
 - Host: transpose + downcast weights to fp16 (layout/dtype only; ternary
   threshold classification verified numerically: rel err ~1.2% < 2e-2),
   x to bf16, shard tokens 1576/core. y returned bf16, upcast on host.
 - Device (per core, identical program):
     * w1 streams in 6 full-kd fp16 chunks (6KB DMA lines - smaller
       chunks are descriptor-bound); each chunk's |row| sums run split
       across DVE (tensor_reduce) and ACT (Abs + accum_out) in parallel
       so the reduce chain tracks the DMA; gamma ends in reciprocal(g).
     * ternary quant in TWO tensor_scalar ops per chunk: w*(1/g) -> int16
       (the HW convert rounds to nearest-even, matching jnp.round), then
       clip to [-1,1] -> fp8. No slow tensor_tensor combine.
     * fc1 phase A is chunk-major with quarter-granularity quant: six
       PSUM accumulation groups stay open and the PE starts right after
       the first quantized quarter-chunk. ~80 junk matmuls keep the PE
       clock (DVFS ramp) at full speed through the DMA phase.
     * fc2: PE matmuls fp8 lhsT x bf16 h; DVE epilogue -> bf16 y out.
     * w2 loads once (fp16, host pre-swizzled [128, 24, 768]), reduced
       and quantized in fc1's slack window; no second pass.
 - PE floor is ~189us (bf16 moving operand); everything else is
   scheduled to keep the PE gapless.
"""
import sys

for _p in ("/root/.axon_site", "/root/.axon_site/_ro/trn_rl_repo",
           "/root/.axon_site/_ro/pypackages", "/opt/trn_rl_repo"):
    if _p not in sys.path:
        sys.path.append(_p)

import ml_dtypes
import numpy as np

from concourse import bacc
import concourse.mybir as mybir
from concourse import bass_isa
from concourse.tile import TileContext

FP32 = mybir.dt.float32
FP16 = mybir.dt.float16
BF16 = mybir.dt.bfloat16
FP8 = mybir.dt.float8e4
I16 = mybir.dt.int16
Act = mybir.ActivationFunctionType
Alu = mybir.AluOpType
AxX = mybir.AxisListType.X

N_CORES = 8
B, S, D, H = 64, 197, 768, 3072
TOK = B * S                 # 12608
TOK_PER = TOK // N_CORES    # 1576
NT = 4                      # token tiles per core
TN = TOK_PER // NT          # 394
KD = D // 128               # 6
KH = H // 128               # 24
EPS = 1e-5

W1C = 12                    # w1 chunks [128, 1536]
HC2 = H // 2
W2B = 6                     # w2 batches [128, 4, 768]
WARM_MM = 14                # gated warm matmuls (DVFS ramp cover)
WARM_N = 512                # columns per warm matmul


def build():
    nc = bacc.Bacc("TRN2", target_bir_lowering=False, debug=False)
    xt = nc.declare_dram_parameter("xt", [128, KD, TOK_PER], BF16, isOutput=False)
    wt1 = nc.declare_dram_parameter("wt1", [D, H], FP16, isOutput=False)
    wt2r = nc.declare_dram_parameter("wt2r", [128, KH, D], FP16, isOutput=False)
    b1r = nc.declare_dram_parameter("b1r", [128, KH], FP32, isOutput=False)
    b2r = nc.declare_dram_parameter("b2r", [128, KD], FP32, isOutput=False)
    yt = nc.declare_dram_parameter("yt", [D, TOK_PER], BF16, isOutput=True)

    with TileContext(nc) as tc:
        with (
            tc.tile_pool(name="singles", bufs=1) as singles,
            tc.tile_pool(name="w1p", bufs=KD) as w1p,       # fp16 w1 resident
            tc.tile_pool(name="t1p", bufs=KD) as t1p,       # fp8 ternary w1
            tc.tile_pool(name="w2p", bufs=W2B) as w2p,       # fp16 w2 resident
            tc.tile_pool(name="t2p", bufs=W2B) as t2p,       # fp8 ternary w2
            tc.tile_pool(name="xb", bufs=3) as xbp,         # x bf16 resident
            tc.tile_pool(name="hb", bufs=74) as hbp,         # gelu outputs
            tc.tile_pool(name="scrD", bufs=2) as scrD,       # int16 round scratch
            tc.tile_pool(name="scrA", bufs=2) as scrA,       # fp8 junk for ACT reduce
            tc.tile_pool(name="ysb", bufs=3) as ysbp,
            tc.tile_pool(name="ps", bufs=8, space="PSUM") as psp,
        ):
            # warm the gpsimd custom-op library while w1 streams in
            dmy = singles.tile([128, 1], FP32, tag="dmy")
            nc.gpsimd.memset(dmy, 0.0)
            dmy2 = singles.tile([128, 1], FP32, tag="dmy2")
            nc.gpsimd.partition_all_reduce(dmy2, dmy, channels=128,
                                           reduce_op=bass_isa.ReduceOp.add)

            # PE pre-warm: keep the tensor engine clocked up during the w1
            # DMA phase so real matmuls start at full DVFS speed.
            wlhs = singles.tile([128, 128], FP8, tag="wlhs")
            nc.vector.memset(wlhs, 0.0)
            wrhs = singles.tile([128, WARM_N], BF16, tag="wrhs")
            nc.vector.memset(wrhs, 0.0)
            onesw = singles.tile([128, 128], FP32, tag="onesw")
            nc.vector.memset(onesw, 1.0)
            wps = psp.tile([128, WARM_N], FP32, tag="ps")

            # biases via the gpsimd DMA queue (idle at start)
            b1sb = singles.tile([128, KH], FP32, tag="b1sb")
            nc.gpsimd.dma_start(out=b1sb, in_=b1r[:, :])
            b2sb = singles.tile([128, KD], FP32, tag="b2sb")
            nc.gpsimd.dma_start(out=b2sb, in_=b2r[:, :])

            # ---- w1 DMA (6 full-kd fp16 chunks, 6KB lines) ----
            w1t = []
            acc1 = singles.tile([128, 2 * KD + 2], FP32, tag="acc1")

            def w1_reduce_pair(wf, lo, hi, col):
                # both engines reduce one half each, in parallel
                mid = (lo + hi) // 2
                nc.vector.tensor_reduce(out=acc1[:, col:col + 1],
                                        in_=wf[:, lo:mid],
                                        axis=AxX, op=Alu.add,
                                        apply_absolute_value=True)
                junk = scrA.tile([128, mid - lo], FP8, tag="scrA")
                nc.scalar.activation(junk, wf[:, mid:hi], Act.Abs,
                                     accum_out=acc1[:, col + 1:col + 2])

            for kd in range(KD):
                wf = w1p.tile([128, H], FP16, tag="w1")
                if kd < KD - 1:
                    nc.sync.dma_start(out=wf,
                                      in_=wt1[kd * 128:(kd + 1) * 128, :])
                    w1t.append(wf)
                    w1_reduce_pair(wf, 0, H, 2 * kd)
                else:
                    # last chunk in two half-DMAs so its reduces start on the
                    # first half while the second is still in flight
                    nc.sync.dma_start(out=wf[:, 0:HC2],
                                      in_=wt1[kd * 128:(kd + 1) * 128, 0:HC2])
                    w1_reduce_pair(wf, 0, HC2, 2 * kd)
                    nc.sync.dma_start(out=wf[:, HC2:H],
                                      in_=wt1[kd * 128:(kd + 1) * 128, HC2:H])
                    w1t.append(wf)
                    w1_reduce_pair(wf, HC2, H, 2 * kd + 2)

            # PE pre-warm, self-timed: gated on the kd4 w1 chunk arrival so
            # the DMA stream runs without SBUF read contention until the
            # ramp actually needs to start (~5.5us before real matmuls).
            for _ in range(WARM_MM):
                nc.tensor.matmul(wps, wlhs, w1t[KD - 2][:, 0:WARM_N],
                                 start=True, stop=True)

            # ---- x DMA (p-major host layout, three 6.3KB-line DMAs so the
            # first kd pair lands well before phase A consumes it) ----
            xparts = []
            for pp in range(3):
                xbt = xbp.tile([128, 2, TOK_PER], BF16, tag="xb")
                nc.sync.dma_start(out=xbt, in_=xt[:, 2 * pp:2 * pp + 2, :])
                xparts.append(xbt)

            def xb_slice(kd, tok):
                return xparts[kd // 2][:, kd % 2, tok]

            # ---- w2 DMA (6 fp16 batches, after x) ----
            w2t = []
            for bt in range(W2B):
                wf = w2p.tile([128, 4, D], FP16, tag="w2")
                nc.sync.dma_start(out=wf, in_=wt2r[:, 4 * bt:4 * bt + 4, :])
                w2t.append(wf)

            def gamma_chain(acc_cols, total_elems, tag):
                """|w| partial sums -> (g, 1/g) broadcast [128,1] fp32."""
                rsum = singles.tile([128, 1], FP32, tag=tag + "_rs")
                nc.vector.tensor_reduce(out=rsum[:, 0:1], in_=acc_cols,
                                        axis=AxX, op=Alu.add)
                allr = singles.tile([128, 1], FP32, tag=tag + "_ar")
                nc.gpsimd.partition_all_reduce(allr, rsum, channels=128,
                                               reduce_op=bass_isa.ReduceOp.add)
                gf = singles.tile([128, 1], FP32, tag=tag + "_gf")
                nc.vector.tensor_scalar(
                    out=gf, in0=allr, scalar1=1.0 / total_elems,
                    scalar2=EPS, op0=Alu.mult, op1=Alu.add)
                gi = singles.tile([128, 1], FP32, tag=tag + "_gi")
                nc.vector.reciprocal(gi, gf)
                return gf, gi

            # ---- gamma1: PE ones-matmul broadcasts the partition total ----
            rsum1 = singles.tile([128, 1], FP32, tag="g1_rs")
            nc.vector.tensor_reduce(out=rsum1[:, 0:1], in_=acc1,
                                    axis=AxX, op=Alu.add)
            psg = psp.tile([128, 1], FP32, tag="ps", name="psg")
            nc.tensor.matmul(psg, onesw, rsum1, start=True, stop=True)
            for _ in range(6):
                nc.tensor.matmul(wps, wlhs, wrhs, start=True, stop=True)
            g1f = singles.tile([128, 1], FP32, tag="g1_gf")
            nc.vector.tensor_scalar(
                out=g1f, in0=psg, scalar1=1.0 / (D * H),
                scalar2=EPS, op0=Alu.mult, op1=Alu.add)
            g1i = singles.tile([128, 1], FP32, tag="g1_gi")
            nc.vector.reciprocal(g1i, g1f)

            def quant(wf, t, gi, n):
                """t = clip(round(w/g), -1, 1) in fp8 via int16 round."""
                r = scrD.tile([128, n], I16, tag="scrD")
                nc.vector.tensor_scalar(out=r, in0=wf, scalar1=gi[:, 0:1],
                                        scalar2=None, op0=Alu.mult)
                nc.vector.tensor_scalar(out=t, in0=r, scalar1=-1.0,
                                        scalar2=1.0, op0=Alu.max, op1=Alu.min)

            # ---- w1 quant: quarter granularity on the even half so the
            # PE can start on hc0-5 after the first ~1us of quant ----
            t1 = [t1p.tile([128, H], FP8, tag="t1", name=f"t1_{kd}")
                  for kd in range(KD)]
            Q = H // 4

            def quant_part(kd, q0, nq):
                sl = slice(q0 * Q, (q0 + nq) * Q)
                quant(w1t[kd][:, sl], t1[kd][:, sl], g1i, nq * Q)

            for kd in range(KD):
                quant_part(kd, 0, 1)

            def t1_slice(hc, kd):
                return t1[kd][:, hc * 128:(hc + 1) * 128]

            hbt = {t: [None] * KH for t in range(NT)}
            ps_open = {}

            def gelu_block(t, hcs):
                for hc in hcs:
                    ps = ps_open.pop(hc)
                    hbv = hbp.tile([128, TN], BF16, tag="hb")
                    nc.scalar.activation(hbv, ps, Act.Gelu,
                                         bias=b1sb[:, hc:hc + 1],
                                         scale=g1f[:, 0:1])
                    hbt[t][hc] = hbv

            def fc1_chunk_major(t, hcs, chunk_order):
                """Open one psum per hc; each chunk contributes immediately."""
                tok = slice(t * TN, (t + 1) * TN)
                for hc in hcs:
                    ps_open[hc] = psp.tile([128, TN], FP32, tag="ps",
                                           name=f"hps_t{t}_hc{hc}")
                for j, kd in enumerate(chunk_order):
                    for hc in hcs:
                        nc.tensor.matmul(ps_open[hc], t1_slice(hc, kd),
                                         xb_slice(kd, tok),
                                         start=(j == 0), stop=(j == KD - 1))

            def fc1_hc_major(t, hcs):
                tok = slice(t * TN, (t + 1) * TN)
                for hc in hcs:
                    ps = psp.tile([128, TN], FP32, tag="ps")
                    for j in range(KD):
                        nc.tensor.matmul(ps, t1_slice(hc, j),
                                         xb_slice(j, tok),
                                         start=(j == 0), stop=(j == KD - 1))
                    ps_open[hc] = ps
                gelu_block(t, hcs)

            # ---- phase A: chunk-major fc1 t0 hc0-5 over q0 quarters ----
            fc1_chunk_major(0, range(0, 6), range(KD))
            for kd in range(KD):
                quant_part(kd, 1, 1)
            gelu_block(0, range(0, 6))
            # ---- A2: t0 hc6-11 chunk-major over q1 quarters ----
            fc1_chunk_major(0, range(6, 12), range(KD))
            gelu_block(0, range(6, 12))

            # ---- odd half (hc12-23), one op per kd ----
            for kd in range(KD):
                quant_part(kd, 2, 2)

            # ---- C: t1 hc0-11 ----
            fc1_hc_major(1, range(0, 12))
            # ---- D/E: t0 hc12-23 ----
            fc1_hc_major(0, range(12, 24))

            # ---- w2 reduces + gamma2 (DVE reaches here after odd quant) ----
            acc2 = singles.tile([128, KH], FP32, tag="acc2")
            for bt in range(W2B):
                nc.vector.tensor_reduce(out=acc2[:, 4 * bt:4 * bt + 4],
                                        in_=w2t[bt], axis=AxX, op=Alu.add,
                                        apply_absolute_value=True)
            g2f, g2i = gamma_chain(acc2, D * H, "g2")

            # ---- F: t1 hc12-23 ----
            fc1_hc_major(1, range(12, 24))

            # ---- w2 quant (all DVE, int16 round) ----
            t2 = [None] * W2B
            for bt in range(W2B):
                t = t2p.tile([128, 4, D], FP8, tag="t2")
                quant(w2t[bt], t, g2i, 4 * D)
                t2[bt] = t

            # ---- G: fc1 t2 full ----
            fc1_hc_major(2, range(0, KH))

            def fc2(t):
                tok = slice(t * TN, (t + 1) * TN)
                for dc in range(KD):
                    ps2 = psp.tile([128, TN], FP32, tag="ps")
                    for j in range(KH):
                        lhsT = t2[j // 4][:, j % 4, dc * 128:(dc + 1) * 128]
                        nc.tensor.matmul(ps2, lhsT, hbt[t][j],
                                         start=(j == 0), stop=(j == KH - 1))
                    ysb = ysbp.tile([128, TN], BF16, tag="ysb")
                    nc.vector.tensor_scalar(
                        out=ysb, in0=ps2, scalar1=g2f[:, 0:1],
                        scalar2=b2sb[:, dc:dc + 1],
                        op0=Alu.mult, op1=Alu.add)
                    if t == NT - 1 and dc == KD - 1:
                        # final transfer: split by partition halves across two
                        # cheap-drain queues to halve the descriptor-bound tail
                        nc.scalar.dma_start(
                            out=yt[dc * 128:dc * 128 + 64, tok],
                            in_=ysb[0:64, :])
                        nc.sync.dma_start(
                            out=yt[dc * 128 + 64:(dc + 1) * 128, tok],
                            in_=ysb[64:128, :])
                    elif dc % 2 == 0:
                        nc.gpsimd.dma_start(
                            out=yt[dc * 128:(dc + 1) * 128, tok], in_=ysb)
                    else:
                        nc.sync.dma_start(
                            out=yt[dc * 128:(dc + 1) * 128, tok], in_=ysb)
                for kh in range(KH):
                    hbt[t][kh] = None

            # ---- H..L ----
            fc2(0)
            fc1_hc_major(3, range(0, KH))
            fc2(1)
            fc2(2)
            fc2(3)

    nc.compile()
    return nc


_NC = None


def _get_nc():
    global _NC
    if _NC is None:
        _NC = build()
    return _NC


def kernel(x, w1, b1, w2, b2, _trace=False, _trace_kwargs=None):
    from concourse.bass_utils import run_bass_kernel_spmd
    nc = _get_nc()
    x = np.asarray(x, dtype=np.float32)
    w1 = np.asarray(w1, dtype=np.float32)
    b1 = np.asarray(b1, dtype=np.float32)
    w2 = np.asarray(w2, dtype=np.float32)
    b2 = np.asarray(b2, dtype=np.float32)
    x2 = np.ascontiguousarray(x.reshape(TOK, D).T).astype(ml_dtypes.bfloat16)
    # per-core p-major layout [128, KD, TOK_PER] for wide DMA lines
    x3 = x2.reshape(KD, 128, TOK)
    wt1 = np.ascontiguousarray(w1.T).astype(np.float16)        # [768, 3072]
    wt2r = np.ascontiguousarray(
        w2.T.reshape(KH, 128, D).transpose(1, 0, 2)).astype(np.float16)
    b1r = np.ascontiguousarray(b1.reshape(KH, 128).T)          # [128, 24]
    b2r = np.ascontiguousarray(b2.reshape(KD, 128).T)          # [128, 6]
    in_maps = []
    for c in range(N_CORES):
        in_maps.append({
            "xt": np.ascontiguousarray(
                x3[:, :, c * TOK_PER:(c + 1) * TOK_PER].transpose(1, 0, 2)),
            "wt1": wt1, "wt2r": wt2r, "b1r": b1r, "b2r": b2r,
        })
    out = run_bass_kernel_spmd(nc, in_maps, list(range(N_CORES)),
                               trace=_trace, **(_trace_kwargs or {}))
    res = out.results
    yt = np.concatenate([res[c]["yt"].astype(np.float32)
                         for c in range(N_CORES)], axis=1)
    y = np.ascontiguousarray(yt.T).reshape(B, S, D)
    if _trace:
        return y, out
    return y
